# revision 12
# baseline (speedup 1.0000x reference)
"""MoE gate (group-limited top-k routing) as a Bass/Tile kernel for 8 TRN2 cores.

Computes, per token:
  logits = hidden @ W            (K=7168, E=256)
  scores = sigmoid(logits) + bias
  group-limited routing: top-2-sum per group of 32 -> top-4 groups of 8
  top-8 of masked scores, renormalized, * 2.5

Sharding: data-parallel over tokens (1024 tokens/core), W + bias replicated.

Matmul scheme (f16x1): quantize x and W to fp16 and do a single fp16 matmul
per K-chunk with fp32 PSUM accumulation. The logits error is ~3e-3 absolute
(|logits| ~ 5), and the output is only the renormalized top-8 *values*, so
near-tie routing flips cost ~nothing: measured routing L2 rel err ~9e-5.
The x tile is split to fp16 on ACT *before* the PE transpose, so the
transpose streams fp16 (1 pass) instead of fp32 (4 passes). Per K-chunk the
PE does 128 (transpose) + 256 (matmul) fp16 streaming cycles, vs 512+512+256
for the fp32-exact hi/lo scheme -- ~2.8x less PE time, putting the kernel at
the HBM roofline (~36.7 MB/core).
"""

import sys

if "/opt/trn_rl_repo" not in sys.path:
    sys.path.insert(0, "/opt/trn_rl_repo")

import numpy as np

import concourse.bacc as bacc
import concourse.bass as bass
import concourse.mybir as mybir
import concourse.tile as tile
from concourse import bass_utils
from concourse.masks import make_identity

P = 128
TOP_K = 8
N_GROUP = 8
TOPK_GROUP = 4
SCALE = 2.5

N_CORES = 8
TOKENS = 8192
HIDDEN = 7168
EXPERTS = 256


def build_moe_gate(
    tokens_per_core=TOKENS // N_CORES,
    hidden=HIDDEN,
    n_experts=EXPERTS,
):
    KC = hidden // P           # K-chunks of 128
    TT = tokens_per_core // P  # token tiles of 128
    GS = n_experts // N_GROUP  # experts per group
    DB = 8                     # K-chunks per DMA slice (512KB; dispatch-rate bound)
    ND = KC // DB              # DMA slices per token tile
    BATCH = 4                  # K-chunks per transpose/copyback batch
    NB = KC // BATCH           # processing batches per token tile
    f32 = mybir.dt.float32
    f16 = mybir.dt.float16

    nc = bacc.Bacc("TRN2", target_bir_lowering=False, debug=False)
    hs = nc.dram_tensor(
        "hidden_states", [tokens_per_core, hidden], f32, kind="ExternalInput"
    ).ap()
    wk = nc.dram_tensor("kernel", [hidden, n_experts], f32, kind="ExternalInput").ap()
    bias = nc.dram_tensor(
        "e_score_correction_bias", [n_experts], f32, kind="ExternalInput"
    ).ap()
    out = nc.dram_tensor(
        "topk_out", [tokens_per_core, TOP_K], f32, kind="ExternalOutput"
    ).ap()

    with tile.TileContext(nc) as tc:
        with (
            tc.tile_pool(name="const", bufs=1) as cpool,
            tc.tile_pool(name="wstage", bufs=4) as wspool,
            tc.tile_pool(name="hload", bufs=16) as hpool,
            tc.tile_pool(name="warm", bufs=1, space="PSUM") as warmpool,
            tc.tile_pool(name="hsplit", bufs=6) as hspool,
            tc.tile_pool(name="ht", bufs=18) as htpool,
            tc.tile_pool(name="ptr", bufs=3, space="PSUM") as ptpool,
            tc.tile_pool(name="plog", bufs=2, space="PSUM") as plpool,
            tc.tile_pool(name="route", bufs=2) as rpool,
        ):
            identity = cpool.tile([P, P], f16)
            make_identity(nc, identity)

            # HAM warm-up: ~5us of dummy matmuls while the PE would otherwise
            # idle waiting for the weight DMAs. Flips the PE clock gate to
            # 8/8 (2.4 GHz) before real work arrives; without this the whole
            # startup phase runs transposes/matmuls at 1.2 GHz.
            warm_ps = warmpool.tile([P, P], f32)
            for _ in range(48):
                nc.tensor.matmul(warm_ps, lhsT=identity, rhs=identity)

            whi = cpool.tile([P, KC, n_experts], f16)
            wk_view = wk.rearrange("(kc p) e -> p kc e", p=P)

            def load_weight_batch(wb):
                # fp32 stage on the shared sync ring; fp16 convert on DVE
                # (gpsimd's CAST is ~6x slower and becomes the critical path)
                wstage = wspool.tile([P, DB, n_experts], f32)
                nc.sync.dma_start(
                    out=wstage, in_=wk_view[:, wb * DB : (wb + 1) * DB, :]
                )
                nc.vector.tensor_copy(
                    whi[:, wb * DB : (wb + 1) * DB, :], wstage
                )

            # bias is only needed by the first routing epilogue, well into the
            # run
            bias_sb = cpool.tile([P, n_experts], f32)
            bias_bcast = bass.AP(
                tensor=bias.tensor, offset=bias.offset, ap=[[0, P]] + list(bias.ap)
            )
            nc.gpsimd.dma_start(out=bias_sb, in_=bias_bcast)

            # all 8 token tiles' outputs land here; one DMA at the end
            wout_all = cpool.tile([P, TT, TOP_K], f32)

            for t in range(TT):
                # DMA in 512KB slices of DB chunks (dispatch is ~640ns fixed
                # per DMA, so finer slices would cap the feed rate); split to
                # fp16 per DMA slice on ACT.
                hi_dma = []
                for s in range(ND):
                    sl = slice(s * DB * P, (s + 1) * DB * P)
                    htile = hpool.tile([P, DB * P], f32)
                    nc.sync.dma_start(out=htile, in_=hs[t * P : (t + 1) * P, sl])
                    if t == 0:
                        # front-load weight DMAs 2-per-hidden-slice: late
                        # weights gate tile-0 matmuls and clog the pipeline
                        for wb in (2 * s, 2 * s + 1):
                            if wb < ND:
                                load_weight_batch(wb)
                    hi = hspool.tile([P, DB * P], f16)
                    nc.scalar.activation(
                        hi, htile, mybir.ActivationFunctionType.Copy
                    )
                    hi_dma.append(hi)

                logits_ps = plpool.tile([P, n_experts], f32)

                hiT = [None] * NB

                def do_transpose(b):
                    # PE transposes for processing batch b (BATCH chunks),
                    # then PSUM->SBUF copyback on DVE
                    hi = hi_dma[b * BATCH // DB]
                    off = (b * BATCH) % DB
                    tp = ptpool.tile([P, BATCH * P], f16)
                    for j in range(BATCH):
                        nc.tensor.transpose(
                            tp[:, j * P : (j + 1) * P],
                            hi[:, (off + j) * P : (off + j + 1) * P],
                            identity,
                        )
                    ht = htpool.tile([P, BATCH * P], f16)
                    nc.vector.tensor_copy(ht, tp)
                    hiT[b] = ht

                def do_matmuls(b):
                    for j in range(BATCH):
                        k = b * BATCH + j
                        nc.tensor.matmul(
                            logits_ps,
                            lhsT=hiT[b][:, j * P : (j + 1) * P],
                            rhs=whi[:, k, :],
                            start=(k == 0),
                            stop=(k == KC - 1),
                        )

                if t == 0:
                    # Weight phase: hidden arrives slowly (bandwidth shared
                    # with the weight stream), and matmuls are weight-gated.
                    # Do ALL of tile 0's transposes first (so matmul stalls
                    # never head-of-line-block the transposes in the in-order
                    # PE queue), with dummy-matmul filler after each batch so
                    # the PE clock gate never sees a >3.4us idle window and
                    # re-throttles to 1.2 GHz.
                    for b in range(NB):
                        do_transpose(b)
                        for _ in range(14):
                            nc.tensor.matmul(warm_ps, lhsT=identity, rhs=identity)
                    for b in range(NB):
                        do_matmuls(b)
                else:
                    # steady state: transposes for batch b+1 issued ahead of
                    # batch b's matmuls so the PE never stalls on batch b's
                    # PSUM->SBUF copyback
                    do_transpose(0)
                    for b in range(NB):
                        if b + 1 < NB:
                            do_transpose(b + 1)
                        do_matmuls(b)

                # ---- routing epilogue (tokens on partitions) ----
                sc = rpool.tile([P, n_experts], f32)
                nc.scalar.activation(
                    sc, logits_ps, mybir.ActivationFunctionType.Sigmoid
                )
                nc.vector.tensor_add(sc, sc, bias_sb)

                # top-2 sum per group of GS experts
                m8 = rpool.tile([P, N_GROUP * 8], f32)
                for g in range(N_GROUP):
                    nc.vector.max(
                        m8[:, g * 8 : (g + 1) * 8], sc[:, g * GS : (g + 1) * GS]
                    )
                m8v = m8.rearrange("p (g k) -> p g k", k=8)
                gsum = rpool.tile([P, N_GROUP], f32)
                nc.vector.tensor_add(gsum, m8v[:, :, 0], m8v[:, :, 1])

                # top-TOPK_GROUP groups -> per-group 0/1 mask via threshold
                gmax = rpool.tile([P, 8], f32)
                nc.vector.max(gmax, gsum)
                gmask = rpool.tile([P, N_GROUP], f32)
                nc.vector.tensor_scalar(
                    gmask,
                    gsum,
                    gmax[:, TOPK_GROUP - 1 : TOPK_GROUP],
                    None,
                    op0=mybir.AluOpType.is_ge,
                )

                # masked scores = sc * mask (0 where group dropped)
                masked = rpool.tile([P, n_experts], f32)
                nc.vector.tensor_mul(
                    masked.rearrange("p (g e) -> p g e", g=N_GROUP),
                    sc.rearrange("p (g e) -> p g e", g=N_GROUP),
                    gmask[:, :, None].broadcast_to([P, N_GROUP, GS]),
                )

                top8 = rpool.tile([P, TOP_K], f32)
                nc.vector.max(top8, masked)

                dsum = rpool.tile([P, 1], f32)
                nc.vector.reduce_sum(dsum, top8, axis=mybir.AxisListType.X)
                rcp = rpool.tile([P, 1], f32)
                nc.vector.reciprocal(rcp, dsum)
                nc.vector.tensor_scalar(
                    wout_all[:, t, :],
                    top8,
                    rcp,
                    SCALE,
                    op0=mybir.AluOpType.mult,
                    op1=mybir.AluOpType.mult,
                )

            nc.sync.dma_start(
                out=out.rearrange("(tt p) k -> p tt k", p=P), in_=wout_all
            )

    nc.compile()
    return nc


_CACHE = {}


def _built_nc():
    if "nc" not in _CACHE:
        _CACHE["nc"] = build_moe_gate()
    return _CACHE["nc"]


def kernel(hidden_states, kernel, e_score_correction_bias):
    hs = np.ascontiguousarray(np.asarray(hidden_states), dtype=np.float32)
    wk = np.ascontiguousarray(np.asarray(kernel), dtype=np.float32)
    bi = np.ascontiguousarray(np.asarray(e_score_correction_bias), dtype=np.float32)
    assert hs.shape == (TOKENS, HIDDEN) and wk.shape == (HIDDEN, EXPERTS)

    tpc = TOKENS // N_CORES
    nc = _built_nc()
    in_maps = [
        {
            "hidden_states": hs[i * tpc : (i + 1) * tpc],
            "kernel": wk,
            "e_score_correction_bias": bi,
        }
        for i in range(N_CORES)
    ]
    res = bass_utils.run_bass_kernel_spmd(nc, in_maps, core_ids=list(range(N_CORES)))
    return np.concatenate(
        [res.results[i]["topk_out"] for i in range(N_CORES)], axis=0
    )


# revision 16
# speedup vs baseline: 1.0337x; 1.0337x over previous
"""MoE gate (group-limited top-k routing) as a Bass/Tile kernel for 8 TRN2 cores.

Computes, per token:
  logits = hidden @ W            (K=7168, E=256)
  scores = sigmoid(logits) + bias
  group-limited routing: top-2-sum per group of 32 -> top-4 groups of 8
  top-8 of masked scores, renormalized, * 2.5

Sharding: data-parallel over tokens (1024 tokens/core), W + bias replicated.

Matmul scheme (f16x1): quantize x and W to fp16 and do a single fp16 matmul
per K-chunk with fp32 PSUM accumulation. The logits error is ~3e-3 absolute
(|logits| ~ 5), and the output is only the renormalized top-8 *values*, so
near-tie routing flips cost ~nothing: measured routing L2 rel err ~9e-5.
The x tile is split to fp16 on ACT *before* the PE transpose, so the
transpose streams fp16 (1 pass) instead of fp32 (4 passes). Per K-chunk the
PE does 128 (transpose) + 256 (matmul) fp16 streaming cycles, vs 512+512+256
for the fp32-exact hi/lo scheme -- ~2.8x less PE time, putting the kernel at
the HBM roofline (~36.7 MB/core).
"""

import sys

if "/opt/trn_rl_repo" not in sys.path:
    sys.path.insert(0, "/opt/trn_rl_repo")

import numpy as np

import concourse.bacc as bacc
import concourse.bass as bass
import concourse.mybir as mybir
import concourse.tile as tile
from concourse import bass_utils
from concourse.masks import make_identity

P = 128
TOP_K = 8
N_GROUP = 8
TOPK_GROUP = 4
SCALE = 2.5

N_CORES = 8
TOKENS = 8192
HIDDEN = 7168
EXPERTS = 256


def build_moe_gate(
    tokens_per_core=TOKENS // N_CORES,
    hidden=HIDDEN,
    n_experts=EXPERTS,
):
    KC = hidden // P           # K-chunks of 128
    TT = tokens_per_core // P  # token tiles of 128
    GS = n_experts // N_GROUP  # experts per group
    DB = 8                     # K-chunks per DMA slice (512KB; dispatch-rate bound)
    ND = KC // DB              # DMA slices per token tile
    BATCH = 8                  # K-chunks per transpose/copyback batch
    NB = KC // BATCH           # processing batches per token tile
    f32 = mybir.dt.float32
    f16 = mybir.dt.float16

    nc = bacc.Bacc("TRN2", target_bir_lowering=False, debug=False)
    hs = nc.dram_tensor(
        "hidden_states", [tokens_per_core, hidden], f32, kind="ExternalInput"
    ).ap()
    wk = nc.dram_tensor("kernel", [hidden, n_experts], f32, kind="ExternalInput").ap()
    bias = nc.dram_tensor(
        "e_score_correction_bias", [n_experts], f32, kind="ExternalInput"
    ).ap()
    out = nc.dram_tensor(
        "topk_out", [tokens_per_core, TOP_K], f32, kind="ExternalOutput"
    ).ap()

    with tile.TileContext(nc) as tc:
        with (
            tc.tile_pool(name="const", bufs=1) as cpool,
            tc.tile_pool(name="wstage", bufs=4) as wspool,
            tc.tile_pool(name="hload", bufs=16) as hpool,
            tc.tile_pool(name="warm", bufs=1, space="PSUM") as warmpool,
            tc.tile_pool(name="hsplit", bufs=6) as hspool,
            tc.tile_pool(name="ht", bufs=18) as htpool,
            tc.tile_pool(name="ptr", bufs=3, space="PSUM") as ptpool,
            tc.tile_pool(name="plog", bufs=2, space="PSUM") as plpool,
            tc.tile_pool(name="route", bufs=2) as rpool,
        ):
            identity = cpool.tile([P, P], f16)
            make_identity(nc, identity)

            # HAM warm-up: ~5us of dummy matmuls while the PE would otherwise
            # idle waiting for the weight DMAs. Flips the PE clock gate to
            # 8/8 (2.4 GHz) before real work arrives; without this the whole
            # startup phase runs transposes/matmuls at 1.2 GHz.
            warm_ps = warmpool.tile([P, P], f32)
            for _ in range(48):
                nc.tensor.matmul(warm_ps, lhsT=identity, rhs=identity)

            whi = cpool.tile([P, KC, n_experts], f16)
            wk_view = wk.rearrange("(kc p) e -> p kc e", p=P)

            def load_weight_batch(wb):
                # fp32 stage on the shared sync ring; fp16 convert on DVE
                # (gpsimd's CAST is ~6x slower and becomes the critical path)
                wstage = wspool.tile([P, DB, n_experts], f32)
                nc.sync.dma_start(
                    out=wstage, in_=wk_view[:, wb * DB : (wb + 1) * DB, :]
                )
                nc.vector.tensor_copy(
                    whi[:, wb * DB : (wb + 1) * DB, :], wstage
                )

            # bias is only needed by the first routing epilogue, well into the
            # run
            bias_sb = cpool.tile([P, n_experts], f32)
            bias_bcast = bass.AP(
                tensor=bias.tensor, offset=bias.offset, ap=[[0, P]] + list(bias.ap)
            )
            nc.gpsimd.dma_start(out=bias_sb, in_=bias_bcast)

            # all 8 token tiles' outputs land here; one DMA at the end
            wout_all = cpool.tile([P, TT, TOP_K], f32)

            for t in range(TT):
                # DMA in 512KB slices of DB chunks (dispatch is ~640ns fixed
                # per DMA, so finer slices would cap the feed rate); split to
                # fp16 per DMA slice on ACT.
                hi_dma = []
                for s in range(ND):
                    sl = slice(s * DB * P, (s + 1) * DB * P)
                    htile = hpool.tile([P, DB * P], f32)
                    nc.sync.dma_start(out=htile, in_=hs[t * P : (t + 1) * P, sl])
                    if t == 0 and s >= 1:
                        # weave ONE weight DMA between hidden slices: keeps
                        # tile-0 transposes streaming with <2.5us gaps (the
                        # PE clock gate re-throttles after ~3.4us idle) while
                        # the weights trickle in for the matmul phase
                        load_weight_batch(s - 1)
                    hi = hspool.tile([P, DB * P], f16)
                    nc.scalar.activation(
                        hi, htile, mybir.ActivationFunctionType.Copy
                    )
                    hi_dma.append(hi)
                if t == 0:
                    load_weight_batch(ND - 1)

                logits_ps = plpool.tile([P, n_experts], f32)

                hiT = [None] * NB

                def do_transpose(b):
                    # PE transposes for processing batch b (BATCH chunks),
                    # then PSUM->SBUF copyback on DVE in two halves so the
                    # first half's matmuls never wait on the second half
                    hi = hi_dma[b * BATCH // DB]
                    off = (b * BATCH) % DB
                    tp = ptpool.tile([P, BATCH * P], f16)
                    for j in range(BATCH):
                        nc.tensor.transpose(
                            tp[:, j * P : (j + 1) * P],
                            hi[:, (off + j) * P : (off + j + 1) * P],
                            identity,
                        )
                    ht = htpool.tile([P, BATCH * P], f16)
                    half = BATCH * P // 2
                    nc.vector.tensor_copy(ht[:, :half], tp[:, :half])
                    nc.vector.tensor_copy(ht[:, half:], tp[:, half:])
                    hiT[b] = ht

                def do_matmuls(b):
                    for j in range(BATCH):
                        k = b * BATCH + j
                        nc.tensor.matmul(
                            logits_ps,
                            lhsT=hiT[b][:, j * P : (j + 1) * P],
                            rhs=whi[:, k, :],
                            start=(k == 0),
                            stop=(k == KC - 1),
                        )

                if t == 0:
                    # Weight phase: matmuls are weight-gated, so do ALL of
                    # tile 0's transposes first (matmul stalls then never
                    # head-of-line-block the transposes in the in-order PE
                    # queue), then the matmuls as weights land.
                    for b in range(NB):
                        do_transpose(b)
                    for b in range(NB):
                        do_matmuls(b)
                else:
                    # steady state: transposes run two batches ahead of the
                    # matmuls so the PE never stalls on a batch's PSUM->SBUF
                    # copyback
                    do_transpose(0)
                    do_transpose(1)
                    for b in range(NB):
                        if b + 2 < NB:
                            do_transpose(b + 2)
                        do_matmuls(b)

                # ---- routing epilogue (tokens on partitions) ----
                sc = rpool.tile([P, n_experts], f32)
                nc.scalar.activation(
                    sc, logits_ps, mybir.ActivationFunctionType.Sigmoid
                )
                nc.vector.tensor_add(sc, sc, bias_sb)

                # top-2 sum per group of GS experts
                m8 = rpool.tile([P, N_GROUP * 8], f32)
                for g in range(N_GROUP):
                    nc.vector.max(
                        m8[:, g * 8 : (g + 1) * 8], sc[:, g * GS : (g + 1) * GS]
                    )
                m8v = m8.rearrange("p (g k) -> p g k", k=8)
                gsum = rpool.tile([P, N_GROUP], f32)
                nc.vector.tensor_add(gsum, m8v[:, :, 0], m8v[:, :, 1])

                # top-TOPK_GROUP groups -> per-group 0/1 mask via threshold
                gmax = rpool.tile([P, 8], f32)
                nc.vector.max(gmax, gsum)
                gmask = rpool.tile([P, N_GROUP], f32)
                nc.vector.tensor_scalar(
                    gmask,
                    gsum,
                    gmax[:, TOPK_GROUP - 1 : TOPK_GROUP],
                    None,
                    op0=mybir.AluOpType.is_ge,
                )

                # masked scores = sc * mask (0 where group dropped)
                masked = rpool.tile([P, n_experts], f32)
                nc.vector.tensor_mul(
                    masked.rearrange("p (g e) -> p g e", g=N_GROUP),
                    sc.rearrange("p (g e) -> p g e", g=N_GROUP),
                    gmask[:, :, None].broadcast_to([P, N_GROUP, GS]),
                )

                top8 = rpool.tile([P, TOP_K], f32)
                nc.vector.max(top8, masked)

                dsum = rpool.tile([P, 1], f32)
                nc.vector.reduce_sum(dsum, top8, axis=mybir.AxisListType.X)
                rcp = rpool.tile([P, 1], f32)
                nc.vector.reciprocal(rcp, dsum)
                nc.vector.tensor_scalar(
                    wout_all[:, t, :],
                    top8,
                    rcp,
                    SCALE,
                    op0=mybir.AluOpType.mult,
                    op1=mybir.AluOpType.mult,
                )

            nc.sync.dma_start(
                out=out.rearrange("(tt p) k -> p tt k", p=P), in_=wout_all
            )

    nc.compile()
    return nc


_CACHE = {}


def _built_nc():
    if "nc" not in _CACHE:
        _CACHE["nc"] = build_moe_gate()
    return _CACHE["nc"]


def kernel(hidden_states, kernel, e_score_correction_bias):
    hs = np.ascontiguousarray(np.asarray(hidden_states), dtype=np.float32)
    wk = np.ascontiguousarray(np.asarray(kernel), dtype=np.float32)
    bi = np.ascontiguousarray(np.asarray(e_score_correction_bias), dtype=np.float32)
    assert hs.shape == (TOKENS, HIDDEN) and wk.shape == (HIDDEN, EXPERTS)

    tpc = TOKENS // N_CORES
    nc = _built_nc()
    in_maps = [
        {
            "hidden_states": hs[i * tpc : (i + 1) * tpc],
            "kernel": wk,
            "e_score_correction_bias": bi,
        }
        for i in range(N_CORES)
    ]
    res = bass_utils.run_bass_kernel_spmd(nc, in_maps, core_ids=list(range(N_CORES)))
    return np.concatenate(
        [res.results[i]["topk_out"] for i in range(N_CORES)], axis=0
    )
